# revision 21
# baseline (speedup 1.0000x reference)
"""Cosine multihead attention on 8 Trainium2 NeuronCores.

Sharding: batch*heads across cores. Core c handles batch b = c // 4 and the
4 heads [4*(c%4), 4*(c%4)+4). Each core computes its heads' q/k/v projections
(tensor-parallel slices of in_proj), full attention for its (B,H) slice, and a
partial out-projection (rank-256 contribution). The host sums the 4 partials
per batch and adds out_proj_bias.

Schedule (v2):
- Inputs land via few large rearranged DMAs spread over 4 engine queues so
  compute starts ~3us in and DMA fully overlaps the projection phase.
- All projections (q, k both head-pairs, then v) run first; q/k bias is folded
  into the PSUM drain (tensor_scalar add), killing the bias matmuls.
- Attention runs qb-outer / pair-inner / kc-inner, software-pipelined with the
  QK matmuls emitted one iteration ahead so the ScalarE exp stream (the
  bottleneck: 128 x ~1.1us activations) never waits on the PE.
- Out-projection matmuls are pumped one-per-iteration as PE fillers inside the
  next qb's attention; only the last qb's out-projection trails the loop.
- All sqrts precede all exps: exactly two ACT table loads.
"""

import sys

if "/opt/trn_rl_repo" not in sys.path:
    sys.path.insert(0, "/opt/trn_rl_repo")

from collections import deque

import numpy as np
import ml_dtypes

import concourse.bass as bass
import concourse.tile as tile
from concourse import bacc, mybir
from concourse.bass_utils import run_bass_kernel_spmd

S, B, E, H = 2048, 2, 1024, 16
HD = E // H            # 64
HPC = 4                # heads per core
NCORES = 8
TAU_MIN = 0.01

BF16 = ml_dtypes.bfloat16
DT_BF = mybir.dt.bfloat16
DT_F32 = mybir.dt.float32

KC_E = E // 128        # 8 contraction chunks for projections
MQ = S // 128          # 16 key chunks of 128
NQB = S // 512         # 4 query blocks of 512
NPAIR = HPC // 2       # 2 head pairs per core


def build_program():
    nc = bacc.Bacc(None)

    xq = nc.dram_tensor("xq_t", [E, S], DT_BF, kind="ExternalInput")
    xk = nc.dram_tensor("xk_t", [E, S], DT_BF, kind="ExternalInput")
    xv = nc.dram_tensor("xv_t", [E, S], DT_BF, kind="ExternalInput")
    # weights arrive pre-arranged p-major ([p, c, n]) so their DMAs are
    # fully contiguous 4KB-per-partition lines.
    wq = nc.dram_tensor("wq_t", [128, KC_E * 256], DT_BF, kind="ExternalInput")
    wk = nc.dram_tensor("wk_t", [128, KC_E * 256], DT_BF, kind="ExternalInput")
    wv = nc.dram_tensor("wv_t", [128, KC_E * 256], DT_BF, kind="ExternalInput")
    bqt = nc.dram_tensor("bq_t", [128, 2], DT_F32, kind="ExternalInput")
    bkt = nc.dram_tensor("bk_t", [128, 2], DT_F32, kind="ExternalInput")
    bv = nc.dram_tensor("b_v", [1, 256], DT_BF, kind="ExternalInput")
    wo = nc.dram_tensor("wo_t", [128, 2 * E], DT_BF, kind="ExternalInput")
    selk_in = nc.dram_tensor("selk", [2, 256], DT_BF, kind="ExternalInput")
    selq_in = nc.dram_tensor("selq", [2, 128], DT_BF, kind="ExternalInput")
    outp = nc.dram_tensor("out_p", [S, E], DT_BF, kind="ExternalOutput")

    with tile.TileContext(nc) as tc:
        with (
            tc.tile_pool(name="consts", bufs=1) as consts,
            tc.tile_pool(name="xin", bufs=1) as xin,
            tc.tile_pool(name="wts", bufs=1) as wts,
            tc.tile_pool(name="qk", bufs=1) as qkpool,
            tc.tile_pool(name="vsb", bufs=1) as vpool,
            tc.tile_pool(name="work", bufs=2) as work,
            tc.tile_pool(name="sqp", bufs=4) as sqp,
            tc.tile_pool(name="outs", bufs=3) as outs,
            tc.tile_pool(name="ps_mm", bufs=2, space="PSUM") as ps_mm,
            tc.tile_pool(name="ps_acc", bufs=3, space="PSUM") as ps_acc,
            tc.tile_pool(name="ps_aux", bufs=1, space="PSUM") as ps_aux,
        ):
            # ---- weights / consts: contiguous p-major DMAs on the scalar
            # ring (fast, ~2.5us total), earliest-need first.
            wq_sb = wts.tile([128, KC_E, 256], DT_BF, tag="wq")
            nc.scalar.dma_start(
                out=wq_sb, in_=wq[:, :].rearrange("p (c n) -> p c n", c=KC_E))
            bq_sb = consts.tile([128, 2], DT_F32, tag="bq")
            nc.scalar.dma_start(out=bq_sb, in_=bqt[:, :])
            selq = consts.tile([2, 128], DT_BF, tag="selq")
            nc.scalar.dma_start(out=selq, in_=selq_in[:, :])
            wk_sb = wts.tile([128, KC_E, 256], DT_BF, tag="wk")
            nc.scalar.dma_start(
                out=wk_sb, in_=wk[:, :].rearrange("p (c n) -> p c n", c=KC_E))
            bk_sb = consts.tile([128, 2], DT_F32, tag="bk")
            nc.scalar.dma_start(out=bk_sb, in_=bkt[:, :])
            selk_sb = consts.tile([2, 256], DT_BF, tag="selk")
            nc.scalar.dma_start(out=selk_sb, in_=selk_in[:, :])
            wv_sb = wts.tile([128, KC_E, 256], DT_BF, tag="wv")
            nc.scalar.dma_start(
                out=wv_sb, in_=wv[:, :].rearrange("p (c n) -> p c n", c=KC_E))
            bv_sb = consts.tile([1, 256], DT_BF, tag="bv")
            nc.scalar.dma_start(out=bv_sb, in_=bv[:, :])
            wo_sb = wts.tile([128, 2, E], DT_BF, tag="wo")
            nc.scalar.dma_start(
                out=wo_sb, in_=wo[:, :].rearrange("p (c n) -> p c n", c=2))

            ones_row = consts.tile([1, 512], DT_BF, tag="ones_row")
            nc.vector.memset(ones_row, 1.0)
            ones_hi = consts.tile([128, 64], DT_F32, tag="ones_hi")
            nc.vector.memset(ones_hi, 1.0)
            hsel = consts.tile([128, 2], DT_BF, tag="hsel")
            nc.vector.memset(hsel, 0.0)
            nc.vector.memset(hsel[0:64, 0:1], 1.0)
            nc.vector.memset(hsel[64:128, 1:2], 1.0)

            # ---- activations: each tensor's 4 chunk-group DMAs spread over
            # all three DMA rings (sync/gpsimd/scalar) so per-tensor
            # completion is ~3x faster; tensor order xq -> xk -> xv matches
            # consumption order.
            xq_sb = xin.tile([128, KC_E, S], DT_BF, tag="xq")
            xk_sb = xin.tile([128, KC_E, S], DT_BF, tag="xk")
            xv_sb = xin.tile([128, KC_E, S], DT_BF, tag="xv")
            nc.sync.dma_start(
                out=xq_sb[:, 0:1, :],
                in_=xq[0:128, :].rearrange("(c p) s -> p c s", p=128))
            nc.gpsimd.dma_start(
                out=xq_sb[:, 1:2, :],
                in_=xq[128:256, :].rearrange("(c p) s -> p c s", p=128))
            for t_sb, t_dram, g0, rings in (
                    (xq_sb, xq, 1, (None, nc.gpsimd, nc.sync, nc.gpsimd)),
                    (xv_sb, xv, 0, (nc.gpsimd, nc.sync, nc.gpsimd, nc.sync)),
                    (xk_sb, xk, 0, (nc.scalar, nc.sync, nc.scalar,
                                    nc.gpsimd))):
                for g in range(g0, 4):
                    src = t_dram[g * 256:(g + 1) * 256, :].rearrange(
                        "(c p) s -> p c s", p=128)
                    rings[g].dma_start(
                        out=t_sb[:, 2 * g:2 * g + 2, :], in_=src)

            qt = [qkpool.tile([128, S], DT_BF, tag=f"qt{p}", name=f"qt{p}")
                  for p in range(NPAIR)]
            kt = [qkpool.tile([128, S], DT_BF, tag=f"kt{p}", name=f"kt{p}")
                  for p in range(NPAIR)]
            heads_t = [qkpool.tile([128, S], DT_BF, tag=f"ht{p}", name=f"ht{p}")
                       for p in range(NPAIR)]

            # ---- q/k projection: 8 accum matmuls per unit, bias folded into
            # the PSUM drain. The L2-norm is pipelined two units deep so the
            # PE never waits on the ACT sqrt: unit u's proj matmuls run while
            # u-1's sumsq->sqrt and u-2's broadcast->scale complete.
            def proj_mms(dst, w_sb, b_col, x_sb, mc, n4):
                sl = slice(n4 * 512, (n4 + 1) * 512)
                pp = ps_mm.tile([128, 512], DT_F32, tag="sc", name="pp_t")
                for c in range(KC_E):
                    nc.tensor.matmul(
                        pp,
                        lhsT=w_sb[:, c, mc * 128:(mc + 1) * 128],
                        rhs=x_sb[:, c, sl],
                        start=(c == 0),
                        stop=(c == KC_E - 1),
                    )
                # drain + bias on ACT (Identity folds the per-partition bias)
                nc.scalar.activation(
                    dst[:, sl], pp, mybir.ActivationFunctionType.Identity,
                    bias=b_col)
                sq = sqp.tile([128, 512], DT_BF, tag="sq", name="sq_t")
                nc.gpsimd.tensor_mul(sq, dst[:, sl], dst[:, sl])
                return (dst, sl, sq)

            def norm_a(stg):
                dst, sl, sq = stg
                ss = ps_acc.tile([2, 512], DT_F32, tag="oacc", name="ss_t")
                nc.tensor.matmul(ss, lhsT=hsel, rhs=sq, start=True, stop=True)
                st = work.tile([2, 512], DT_BF, tag="st", name="st_t")
                nc.scalar.activation(st, ss, mybir.ActivationFunctionType.Sqrt)
                return (dst, sl, st)

            def norm_b(stg, sel):
                dst, sl, st = stg
                rb = ps_aux.tile([128, 512], DT_F32, tag="aux", name="rb_t")
                nc.tensor.matmul(rb, lhsT=sel, rhs=st, start=True, stop=True)
                rbi = work.tile([128, 512], DT_F32, tag="rbi", name="rbi_t")
                nc.vector.reciprocal_approx_fast(out=rbi, in_=rb)
                nc.gpsimd.tensor_mul(dst[:, sl], dst[:, sl], rbi)

            q_units = [
                (qt[mc], wq_sb, bq_sb[:, mc:mc + 1], xq_sb, selq, mc, n4)
                for mc in range(NPAIR) for n4 in range(4)
            ]
            k_units = [
                (kt[mc], wk_sb, bk_sb[:, mc:mc + 1], xk_sb,
                 selk_sb[:, mc * 128:(mc + 1) * 128], mc, n4)
                for mc in range(NPAIR) for n4 in range(4)
            ]
            q_a, q_b = deque(), deque()

            def pump_unit(unit):
                dst, w_sb, b_col, x_sb, sel, mc, n4 = unit
                stg = proj_mms(dst, w_sb, b_col, x_sb, mc, n4)
                q_a.append((stg, sel))
                if len(q_a) >= 2:
                    s, sel_ = q_a.popleft()
                    q_b.append((norm_a(s), sel_))
                if len(q_b) >= 2:
                    s, sel_ = q_b.popleft()
                    norm_b(s, sel_)

            def flush_units():
                while q_a:
                    s, sel_ = q_a.popleft()
                    q_b.append((norm_a(s), sel_))
                while q_b:
                    s, sel_ = q_b.popleft()
                    norm_b(s, sel_)

            for unit in q_units:
                pump_unit(unit)

            # ---- v projection between q and k (natural orientation, ones
            # column for the softmax denominator ride-along in PV); the q-norm
            # pipeline flushes underneath the first v matmuls, and attention's
            # progressive kt consumption later overlaps the k-proj tail.
            v_sb = vpool.tile([128, MQ, HPC, HD + 1], DT_BF, tag="v")
            nc.vector.memset(v_sb[:, :, :, HD:HD + 1], 1.0)
            for m in range(MQ):
                vp = ps_acc.tile([128, 256], DT_F32, tag="oacc", name="vp_t")
                for c in range(KC_E):
                    nc.tensor.matmul(
                        vp,
                        lhsT=xv_sb[:, c, m * 128:(m + 1) * 128],
                        rhs=wv_sb[:, c, :],
                        start=(c == 0),
                        stop=False,
                    )
                nc.tensor.matmul(
                    vp,
                    lhsT=ones_row[0:1, 0:128],
                    rhs=bv_sb[0:1, :],
                    start=False,
                    stop=True,
                )
                nc.vector.tensor_copy(
                    out=v_sb[:, m, :, 0:HD],
                    in_=vp.rearrange("p (h d) -> p h d", h=HPC),
                )
                if m < 2 and q_a:
                    s, sel_ = q_a.popleft()
                    q_b.append((norm_a(s), sel_))
                elif q_b:
                    s, sel_ = q_b.popleft()
                    norm_b(s, sel_)

            for unit in k_units:
                pump_unit(unit)
            # drain the sqrt stages now (they must precede the first exp to
            # avoid ACT table thrash); the ACT-free broadcast/scale stages of
            # kt[1]'s last chunks defer past emit_qk(0) so the exp stream
            # starts immediately.
            while q_a:
                s_, sel_ = q_a.popleft()
                q_b.append((norm_a(s_), sel_))

            # ---- attention: software-pipelined over (qb, pair, kc) ---------
            iters = [(qb, p, kc)
                     for qb in range(NQB) for p in range(NPAIR)
                     for kc in range(MQ)]
            NIT = len(iters)
            sc_t, ex_t, o_t = {}, {}, {}
            fillers = deque()

            def emit_qk(j):
                qb, p, kc = iters[j]
                sl_q = slice(qb * 512, (qb + 1) * 512)
                sc = ps_mm.tile([128, 1024], DT_F32, tag="sc", name="sc_t")
                nc.tensor.matmul(
                    sc[:, 0:512],
                    lhsT=kt[p][0:64, kc * 128:(kc + 1) * 128],
                    rhs=qt[p][0:64, sl_q],
                    start=True, stop=True,
                )
                nc.tensor.matmul(
                    sc[:, 512:1024],
                    lhsT=kt[p][64:128, kc * 128:(kc + 1) * 128],
                    rhs=qt[p][64:128, sl_q],
                    start=True, stop=True,
                )
                sc_t[j] = sc

            def emit_exp(j):
                sc = sc_t.pop(j)
                ex = work.tile([128, 1024], DT_BF, tag="exp", name="ex_t")
                nc.scalar.activation(ex, sc, mybir.ActivationFunctionType.Exp)
                ex_t[j] = ex

            def emit_pv(j):
                qb, p, kc = iters[j]
                ex = ex_t.pop(j)
                if kc == 0:
                    o_t[0] = ps_acc.tile([128, 512], DT_F32, tag="oacc",
                                         name="o0_t")
                    o_t[1] = ps_acc.tile([128, 512], DT_F32, tag="oacc",
                                         name="o1_t")
                nc.tensor.matmul(
                    o_t[0][0:65, :],
                    lhsT=v_sb[:, kc, 2 * p, :],
                    rhs=ex[:, 0:512],
                    start=(kc == 0), stop=(kc == MQ - 1),
                )
                nc.tensor.matmul(
                    o_t[1][0:65, :],
                    lhsT=v_sb[:, kc, 2 * p + 1, :],
                    rhs=ex[:, 512:1024],
                    start=(kc == 0), stop=(kc == MQ - 1),
                )

            def finish_segment(qb, p):
                """Copy the denominators out now (DVE, cheap); return one
                deferred closure doing broadcast+reciprocal+normalize so the
                PE-side zb matmuls never delay the next segment's QK, and the
                o-slots are freed as early as possible."""
                sl_q = slice(qb * 512, (qb + 1) * 512)
                o0, o1 = o_t[0], o_t[1]
                # one fast copy per head frees the PSUM o-slot immediately;
                # the normalize chain then runs off the SBUF copy at leisure.
                oc0 = work.tile([128, 512], DT_F32, tag="oc", name="oc0_t",
                                bufs=4)
                nc.vector.tensor_copy(oc0[0:65, :], o0[0:65, :])
                oc1 = work.tile([128, 512], DT_F32, tag="oc", name="oc1_t",
                                bufs=4)
                nc.vector.tensor_copy(oc1[0:65, :], o1[0:65, :])

                def go():
                    zb0 = ps_aux.tile([64, 512], DT_F32, tag="aux",
                                      name="zb0_t")
                    nc.tensor.matmul(
                        zb0, lhsT=ones_hi[64:65, 0:64], rhs=oc0[64:65, :],
                        start=True, stop=True)
                    zbi0 = work.tile([64, 512], DT_F32, tag="ot",
                                     name="zbi0_t")
                    nc.vector.reciprocal_approx_fast(out=zbi0, in_=zb0)
                    zb1 = ps_aux.tile([64, 512], DT_F32, tag="aux",
                                      name="zb1_t")
                    nc.tensor.matmul(
                        zb1, lhsT=ones_hi[64:65, 0:64], rhs=oc1[64:65, :],
                        start=True, stop=True)
                    nc.gpsimd.tensor_mul(
                        heads_t[p][0:64, sl_q], oc0[0:64, :], zbi0)
                    zbi1 = work.tile([64, 512], DT_F32, tag="ot",
                                     name="zbi1_t")
                    nc.vector.reciprocal_approx_fast(out=zbi1, in_=zb1)
                    t2 = work.tile([64, 512], DT_BF, tag="t2", name="t2_t")
                    nc.gpsimd.tensor_mul(t2, oc1[0:64, :], zbi1)
                    nc.gpsimd.dma_start(
                        out=heads_t[p][64:128, sl_q], in_=t2)

                return go

            def outproj_ops(m, n2):
                sl_n = slice(n2 * 512, (n2 + 1) * 512)
                st8 = {}

                def mm0():
                    st8["op"] = ps_aux.tile([128, 512], DT_F32, tag="aux",
                                            name="op_t")
                    nc.tensor.matmul(
                        st8["op"],
                        lhsT=heads_t[0][:, m * 128:(m + 1) * 128],
                        rhs=wo_sb[:, 0, sl_n],
                        start=True, stop=False,
                    )

                def mm1():
                    nc.tensor.matmul(
                        st8["op"],
                        lhsT=heads_t[1][:, m * 128:(m + 1) * 128],
                        rhs=wo_sb[:, 1, sl_n],
                        start=False, stop=True,
                    )

                def drain():
                    ob = outs.tile([128, 512], DT_BF, tag="ob", name="ob_t")
                    nc.vector.tensor_copy(ob, st8["op"])
                    nc.sync.dma_start(
                        out=outp[m * 128:(m + 1) * 128, sl_n], in_=ob)

                return [mm0, mm1, drain]

            pA = {}

            def mk_partial0(m, n2):
                def go():
                    opp = ps_aux.tile([128, 512], DT_F32, tag="aux",
                                      name="p0_t")
                    nc.tensor.matmul(
                        opp,
                        lhsT=heads_t[0][:, m * 128:(m + 1) * 128],
                        rhs=wo_sb[:, 0, n2 * 512:(n2 + 1) * 512],
                        start=True, stop=True,
                    )
                    pa = outs.tile([128, 512], DT_F32, tag="pa",
                                   name="pa_t", bufs=8)
                    nc.vector.tensor_copy(pa, opp)
                    pA[m, n2] = pa
                return go

            # preload the exp table while v-projection runs so the switch cost
            # is off the attention critical path
            exwarm = work.tile([1, 16], DT_BF, tag="exw", name="exw_t")
            nc.scalar.activation(exwarm, ones_row[0:1, 0:16],
                                 mybir.ActivationFunctionType.Exp)

            emit_qk(0)
            for j in range(NIT):
                qb, p, kc = iters[j]
                emit_exp(j)
                if j == 0:
                    while q_b:
                        s_, sel_ = q_b.popleft()
                        norm_b(s_, sel_)
                if j + 1 < NIT:
                    emit_qk(j + 1)
                if fillers:
                    fillers.popleft()()
                emit_pv(j)
                if kc == MQ - 1:
                    fillers.insert(0, finish_segment(qb, p))
                    if p == NPAIR - 1 and qb < NQB - 1:
                        for m in range(4 * qb, 4 * qb + 4):
                            for n2 in range(2):
                                fillers.extend(outproj_ops(m, n2))
                    elif p == 0 and qb == NQB - 1:
                        # qb3: heads_t[0]-half of the out-projection runs as
                        # fillers during qb3-p1; partials staged in SBUF so
                        # the tail only needs the heads_t[1] matmul + add.
                        for m in range(4 * qb, 4 * qb + 4):
                            for n2 in range(2):
                                fillers.append(mk_partial0(m, n2))
            while fillers:
                fillers.popleft()()

            # last qb's out-projection: dense back-to-back matmuls through the
            # (now idle) double-buffered sc-tag PSUM slots; drains on the idle
            # ACT engine, DMAs split over two rings.
            for m in range(4 * (NQB - 1), 4 * NQB):
                for n2 in range(2):
                    opp = ps_mm.tile([128, 512], DT_F32, tag="sc",
                                     name="opp_t")
                    nc.tensor.matmul(
                        opp,
                        lhsT=heads_t[1][:, m * 128:(m + 1) * 128],
                        rhs=wo_sb[:, 1, n2 * 512:(n2 + 1) * 512],
                        start=True, stop=True,
                    )
                    ob = outs.tile([128, 512], DT_BF, tag="ob", name="ob_t")
                    nc.vector.tensor_add(ob, opp, pA[m, n2])
                    eng = nc.sync if n2 == 0 else nc.gpsimd
                    eng.dma_start(
                        out=outp[m * 128:(m + 1) * 128,
                                 n2 * 512:(n2 + 1) * 512],
                        in_=ob)

    nc.compile()
    return nc


_CACHE = {}


def _get_program():
    if "nc" not in _CACHE:
        _CACHE["nc"] = build_program()
    return _CACHE["nc"]


def _pmajor(w):
    """[C, N] -> [128, (C//128)*N]: chunk c's rows c*128+p land at
    partition p, free offset c*N — contiguous per-partition DMA lines."""
    C, N = w.shape
    return np.ascontiguousarray(
        w.reshape(C // 128, 128, N).transpose(1, 0, 2).reshape(128, -1)
    ).astype(BF16)


def make_in_maps(query, key, value, in_proj_weight, in_proj_bias,
                 out_proj_weight, out_proj_bias, tau):
    query = np.asarray(query, np.float32)
    key = np.asarray(key, np.float32)
    value = np.asarray(value, np.float32)
    W = np.asarray(in_proj_weight, np.float32)
    bias = np.asarray(in_proj_bias, np.float32)
    Wo = np.asarray(out_proj_weight, np.float32)
    tau_c = np.maximum(np.asarray(tau, np.float32).reshape(H), TAU_MIN)

    # Transposed activations per batch: (E, S) bf16
    xT = {}
    for b in range(B):
        xT["q", b] = np.ascontiguousarray(query[:, b, :].T).astype(BF16)
        xT["k", b] = np.ascontiguousarray(key[:, b, :].T).astype(BF16)
        xT["v", b] = np.ascontiguousarray(value[:, b, :].T).astype(BF16)

    selq_host = np.zeros((2, 128), np.float32)
    selq_host[0, 0:64] = 1.0
    selq_host[1, 64:128] = 1.0
    selq_host = selq_host.astype(BF16)
    in_maps = []
    for c in range(NCORES):
        b = c // 4
        h0 = HPC * (c % 4)
        rows = slice(h0 * HD, (h0 + HPC) * HD)
        rows_k = slice(E + h0 * HD, E + (h0 + HPC) * HD)
        rows_v = slice(2 * E + h0 * HD, 2 * E + (h0 + HPC) * HD)
        # per-pair selector with tau folded in on the k side (the subsequent
        # reciprocal turns tau * |k| into 1 / (tau * |k|))
        selk = np.zeros((2, 256), np.float32)
        for mc in range(NPAIR):
            selk[0, mc * 128:mc * 128 + 64] = tau_c[h0 + 2 * mc]
            selk[1, mc * 128 + 64:(mc + 1) * 128] = tau_c[h0 + 2 * mc + 1]
        in_maps.append({
            "xq_t": xT["q", b],
            "xk_t": xT["k", b],
            "xv_t": xT["v", b],
            "wq_t": _pmajor(W[rows, :].T),
            "wk_t": _pmajor(W[rows_k, :].T),
            "wv_t": _pmajor(W[rows_v, :].T),
            "bq_t": np.ascontiguousarray(
                bias[rows].reshape(2, 128).T).astype(np.float32),
            "bk_t": np.ascontiguousarray(
                bias[rows_k].reshape(2, 128).T).astype(np.float32),
            "b_v": bias[rows_v].reshape(1, 256).astype(BF16),
            "wo_t": _pmajor(Wo[:, rows].T),
            "selk": selk.astype(BF16),
            "selq": selq_host,
        })
    return in_maps


def assemble_out(results, out_proj_bias):
    bo = np.asarray(out_proj_bias, np.float32)
    out = np.zeros((S, B, E), np.float32)
    for c in range(NCORES):
        out[:, c // 4, :] += results[c]["out_p"].astype(np.float32)
    out += bo[None, None, :]
    return out


def kernel(query, key, value, in_proj_weight, in_proj_bias,
           out_proj_weight, out_proj_bias, tau):
    nc = _get_program()
    in_maps = make_in_maps(query, key, value, in_proj_weight, in_proj_bias,
                           out_proj_weight, out_proj_bias, tau)
    res = run_bass_kernel_spmd(nc, in_maps, core_ids=list(range(NCORES)))
    return assemble_out(res.results, out_proj_bias)


if __name__ == "__main__":
    import reference

    inputs = {k: np.asarray(v) for k, v in reference.setup_inputs().items()}
    out = kernel(**inputs)
    print("out shape", out.shape, out.dtype)


# revision 23
# speedup vs baseline: 1.0292x; 1.0292x over previous
"""Cosine multihead attention on 8 Trainium2 NeuronCores.

Sharding: batch*heads across cores. Core c handles batch b = c // 4 and the
4 heads [4*(c%4), 4*(c%4)+4). Each core computes its heads' q/k/v projections
(tensor-parallel slices of in_proj), full attention for its (B,H) slice, and a
partial out-projection (rank-256 contribution). The host sums the 4 partials
per batch and adds out_proj_bias.

Schedule (v2):
- Inputs land via few large rearranged DMAs spread over 4 engine queues so
  compute starts ~3us in and DMA fully overlaps the projection phase.
- All projections (q, k both head-pairs, then v) run first; q/k bias is folded
  into the PSUM drain (tensor_scalar add), killing the bias matmuls.
- Attention runs qb-outer / pair-inner / kc-inner, software-pipelined with the
  QK matmuls emitted one iteration ahead so the ScalarE exp stream (the
  bottleneck: 128 x ~1.1us activations) never waits on the PE.
- Out-projection matmuls are pumped one-per-iteration as PE fillers inside the
  next qb's attention; only the last qb's out-projection trails the loop.
- All sqrts precede all exps: exactly two ACT table loads.
"""

import sys

if "/opt/trn_rl_repo" not in sys.path:
    sys.path.insert(0, "/opt/trn_rl_repo")

from collections import deque

import numpy as np
import ml_dtypes

import concourse.bass as bass
import concourse.tile as tile
from concourse import bacc, mybir
from concourse.bass_utils import run_bass_kernel_spmd

S, B, E, H = 2048, 2, 1024, 16
HD = E // H            # 64
HPC = 4                # heads per core
NCORES = 8
TAU_MIN = 0.01

BF16 = ml_dtypes.bfloat16
DT_BF = mybir.dt.bfloat16
DT_F32 = mybir.dt.float32

KC_E = E // 128        # 8 contraction chunks for projections
MQ = S // 128          # 16 key chunks of 128
NQB = S // 512         # 4 query blocks of 512
NPAIR = HPC // 2       # 2 head pairs per core


def build_program():
    nc = bacc.Bacc(None)

    xq = nc.dram_tensor("xq_t", [E, S], DT_BF, kind="ExternalInput")
    xk = nc.dram_tensor("xk_t", [E, S], DT_BF, kind="ExternalInput")
    xv = nc.dram_tensor("xv_t", [E, S], DT_BF, kind="ExternalInput")
    # weights arrive pre-arranged p-major ([p, c, n]) so their DMAs are
    # fully contiguous 4KB-per-partition lines.
    wq = nc.dram_tensor("wq_t", [128, KC_E * 256], DT_BF, kind="ExternalInput")
    wk = nc.dram_tensor("wk_t", [128, KC_E * 256], DT_BF, kind="ExternalInput")
    wv = nc.dram_tensor("wv_t", [128, KC_E * 256], DT_BF, kind="ExternalInput")
    bqt = nc.dram_tensor("bq_t", [128, 2], DT_F32, kind="ExternalInput")
    bkt = nc.dram_tensor("bk_t", [128, 2], DT_F32, kind="ExternalInput")
    bv = nc.dram_tensor("b_v", [1, 256], DT_BF, kind="ExternalInput")
    wo = nc.dram_tensor("wo_t", [128, 2 * E], DT_BF, kind="ExternalInput")
    selk_in = nc.dram_tensor("selk", [2, 256], DT_BF, kind="ExternalInput")
    selq_in = nc.dram_tensor("selq", [2, 128], DT_BF, kind="ExternalInput")
    outp = nc.dram_tensor("out_p", [S, E], DT_BF, kind="ExternalOutput")

    with tile.TileContext(nc) as tc:
        with (
            tc.tile_pool(name="consts", bufs=1) as consts,
            tc.tile_pool(name="xin", bufs=1) as xin,
            tc.tile_pool(name="wts", bufs=1) as wts,
            tc.tile_pool(name="qk", bufs=1) as qkpool,
            tc.tile_pool(name="vsb", bufs=1) as vpool,
            tc.tile_pool(name="work", bufs=2) as work,
            tc.tile_pool(name="sqp", bufs=4) as sqp,
            tc.tile_pool(name="outs", bufs=3) as outs,
            tc.tile_pool(name="ps_mm", bufs=2, space="PSUM") as ps_mm,
            tc.tile_pool(name="ps_acc", bufs=3, space="PSUM") as ps_acc,
            tc.tile_pool(name="ps_aux", bufs=1, space="PSUM") as ps_aux,
        ):
            # ---- weights / consts: contiguous p-major DMAs on the scalar
            # ring (fast, ~2.5us total), earliest-need first.
            wq_sb = wts.tile([128, KC_E, 256], DT_BF, tag="wq")
            nc.scalar.dma_start(
                out=wq_sb, in_=wq[:, :].rearrange("p (c n) -> p c n", c=KC_E))
            bq_sb = consts.tile([128, 2], DT_F32, tag="bq")
            nc.scalar.dma_start(out=bq_sb, in_=bqt[:, :])
            selq = consts.tile([2, 128], DT_BF, tag="selq")
            nc.scalar.dma_start(out=selq, in_=selq_in[:, :])
            wk_sb = wts.tile([128, KC_E, 256], DT_BF, tag="wk")
            nc.scalar.dma_start(
                out=wk_sb, in_=wk[:, :].rearrange("p (c n) -> p c n", c=KC_E))
            bk_sb = consts.tile([128, 2], DT_F32, tag="bk")
            nc.scalar.dma_start(out=bk_sb, in_=bkt[:, :])
            selk_sb = consts.tile([2, 256], DT_BF, tag="selk")
            nc.scalar.dma_start(out=selk_sb, in_=selk_in[:, :])
            wv_sb = wts.tile([128, KC_E, 256], DT_BF, tag="wv")
            nc.scalar.dma_start(
                out=wv_sb, in_=wv[:, :].rearrange("p (c n) -> p c n", c=KC_E))
            bv_sb = consts.tile([1, 256], DT_BF, tag="bv")
            nc.scalar.dma_start(out=bv_sb, in_=bv[:, :])
            wo_sb = wts.tile([128, 2, E], DT_BF, tag="wo")
            nc.scalar.dma_start(
                out=wo_sb, in_=wo[:, :].rearrange("p (c n) -> p c n", c=2))

            ones_row = consts.tile([1, 512], DT_BF, tag="ones_row")
            nc.vector.memset(ones_row, 1.0)
            ones_hi = consts.tile([128, 64], DT_F32, tag="ones_hi")
            nc.vector.memset(ones_hi, 1.0)
            hsel = consts.tile([128, 2], DT_BF, tag="hsel")
            nc.vector.memset(hsel, 0.0)
            nc.vector.memset(hsel[0:64, 0:1], 1.0)
            nc.vector.memset(hsel[64:128, 1:2], 1.0)

            # ---- activations: each tensor's 4 chunk-group DMAs spread over
            # all three DMA rings (sync/gpsimd/scalar) so per-tensor
            # completion is ~3x faster; tensor order xq -> xk -> xv matches
            # consumption order.
            xq_sb = xin.tile([128, KC_E, S], DT_BF, tag="xq")
            xk_sb = xin.tile([128, KC_E, S], DT_BF, tag="xk")
            xv_sb = xin.tile([128, KC_E, S], DT_BF, tag="xv")
            for t_sb, t_dram, g0, rings in (
                    (xq_sb, xq, 0, (nc.sync, nc.gpsimd, nc.sync, nc.gpsimd)),
                    (xv_sb, xv, 0, (nc.gpsimd, nc.sync, nc.gpsimd, nc.sync)),
                    (xk_sb, xk, 0, (nc.scalar, nc.sync, nc.scalar,
                                    nc.gpsimd))):
                for g in range(g0, 4):
                    src = t_dram[g * 256:(g + 1) * 256, :].rearrange(
                        "(c p) s -> p c s", p=128)
                    rings[g].dma_start(
                        out=t_sb[:, 2 * g:2 * g + 2, :], in_=src)

            qt = [qkpool.tile([128, S], DT_BF, tag=f"qt{p}", name=f"qt{p}")
                  for p in range(NPAIR)]
            kt = [qkpool.tile([128, S], DT_BF, tag=f"kt{p}", name=f"kt{p}")
                  for p in range(NPAIR)]
            heads_t = [qkpool.tile([128, S], DT_BF, tag=f"ht{p}", name=f"ht{p}")
                       for p in range(NPAIR)]

            # ---- q/k projection: 8 accum matmuls per unit, bias folded into
            # the PSUM drain. The L2-norm is pipelined two units deep so the
            # PE never waits on the ACT sqrt: unit u's proj matmuls run while
            # u-1's sumsq->sqrt and u-2's broadcast->scale complete.
            def proj_mms(dst, w_sb, b_col, x_sb, mc, n4):
                sl = slice(n4 * 512, (n4 + 1) * 512)
                pp = ps_mm.tile([128, 512], DT_F32, tag="sc", name="pp_t")
                for c in range(KC_E):
                    nc.tensor.matmul(
                        pp,
                        lhsT=w_sb[:, c, mc * 128:(mc + 1) * 128],
                        rhs=x_sb[:, c, sl],
                        start=(c == 0),
                        stop=(c == KC_E - 1),
                    )
                # drain + bias on ACT (Identity folds the per-partition bias)
                nc.scalar.activation(
                    dst[:, sl], pp, mybir.ActivationFunctionType.Identity,
                    bias=b_col)
                sq = sqp.tile([128, 512], DT_BF, tag="sq", name="sq_t")
                nc.gpsimd.tensor_mul(sq, dst[:, sl], dst[:, sl])
                return (dst, sl, sq)

            def norm_a(stg):
                dst, sl, sq = stg
                ss = ps_acc.tile([2, 512], DT_F32, tag="oacc", name="ss_t")
                nc.tensor.matmul(ss, lhsT=hsel, rhs=sq, start=True, stop=True)
                st = work.tile([2, 512], DT_BF, tag="st", name="st_t")
                nc.scalar.activation(st, ss, mybir.ActivationFunctionType.Sqrt)
                return (dst, sl, st)

            def norm_b(stg, sel):
                dst, sl, st = stg
                rb = ps_aux.tile([128, 512], DT_F32, tag="aux", name="rb_t")
                nc.tensor.matmul(rb, lhsT=sel, rhs=st, start=True, stop=True)
                rbi = work.tile([128, 512], DT_F32, tag="rbi", name="rbi_t")
                nc.vector.reciprocal_approx_fast(out=rbi, in_=rb)
                nc.gpsimd.tensor_mul(dst[:, sl], dst[:, sl], rbi)

            q_units = [
                (qt[mc], wq_sb, bq_sb[:, mc:mc + 1], xq_sb, selq, mc, n4)
                for mc in range(NPAIR) for n4 in range(4)
            ]
            k_units = [
                (kt[mc], wk_sb, bk_sb[:, mc:mc + 1], xk_sb,
                 selk_sb[:, mc * 128:(mc + 1) * 128], mc, n4)
                for mc in range(NPAIR) for n4 in range(4)
            ]
            q_a, q_b = deque(), deque()

            def pump_unit(unit):
                dst, w_sb, b_col, x_sb, sel, mc, n4 = unit
                stg = proj_mms(dst, w_sb, b_col, x_sb, mc, n4)
                q_a.append((stg, sel))
                if len(q_a) >= 2:
                    s, sel_ = q_a.popleft()
                    q_b.append((norm_a(s), sel_))
                if len(q_b) >= 2:
                    s, sel_ = q_b.popleft()
                    norm_b(s, sel_)

            def flush_units():
                while q_a:
                    s, sel_ = q_a.popleft()
                    q_b.append((norm_a(s), sel_))
                while q_b:
                    s, sel_ = q_b.popleft()
                    norm_b(s, sel_)

            for unit in q_units:
                pump_unit(unit)

            # ---- v projection between q and k (natural orientation, ones
            # column for the softmax denominator ride-along in PV); the q-norm
            # pipeline flushes underneath the first v matmuls, and attention's
            # progressive kt consumption later overlaps the k-proj tail.
            v_sb = vpool.tile([128, MQ, HPC, HD + 1], DT_BF, tag="v")
            nc.vector.memset(v_sb[:, :, :, HD:HD + 1], 1.0)
            for m in range(MQ):
                vp = ps_acc.tile([128, 256], DT_F32, tag="oacc", name="vp_t")
                for c in range(KC_E):
                    nc.tensor.matmul(
                        vp,
                        lhsT=xv_sb[:, c, m * 128:(m + 1) * 128],
                        rhs=wv_sb[:, c, :],
                        start=(c == 0),
                        stop=False,
                    )
                nc.tensor.matmul(
                    vp,
                    lhsT=ones_row[0:1, 0:128],
                    rhs=bv_sb[0:1, :],
                    start=False,
                    stop=True,
                )
                nc.vector.tensor_copy(
                    out=v_sb[:, m, :, 0:HD],
                    in_=vp.rearrange("p (h d) -> p h d", h=HPC),
                )
                if m < 2 and q_a:
                    s, sel_ = q_a.popleft()
                    q_b.append((norm_a(s), sel_))
                elif q_b:
                    s, sel_ = q_b.popleft()
                    norm_b(s, sel_)

            for unit in k_units:
                pump_unit(unit)
            # drain the sqrt stages now (they must precede the first exp to
            # avoid ACT table thrash); the ACT-free broadcast/scale stages of
            # kt[1]'s last chunks defer past emit_qk(0) so the exp stream
            # starts immediately.
            while q_a:
                s_, sel_ = q_a.popleft()
                q_b.append((norm_a(s_), sel_))

            # ---- attention: software-pipelined over (qb, pair, kc) ---------
            iters = [(qb, p, kc)
                     for qb in range(NQB) for p in range(NPAIR)
                     for kc in range(MQ)]
            NIT = len(iters)
            sc_t, ex_t, o_t = {}, {}, {}
            fillers = deque()

            def emit_qk(j):
                qb, p, kc = iters[j]
                sl_q = slice(qb * 512, (qb + 1) * 512)
                sc = ps_mm.tile([128, 1024], DT_F32, tag="sc", name="sc_t")
                nc.tensor.matmul(
                    sc[:, 0:512],
                    lhsT=kt[p][0:64, kc * 128:(kc + 1) * 128],
                    rhs=qt[p][0:64, sl_q],
                    start=True, stop=True,
                )
                nc.tensor.matmul(
                    sc[:, 512:1024],
                    lhsT=kt[p][64:128, kc * 128:(kc + 1) * 128],
                    rhs=qt[p][64:128, sl_q],
                    start=True, stop=True,
                )
                sc_t[j] = sc

            def emit_exp(j):
                sc = sc_t.pop(j)
                ex = work.tile([128, 1024], DT_BF, tag="exp", name="ex_t")
                nc.scalar.activation(ex, sc, mybir.ActivationFunctionType.Exp)
                ex_t[j] = ex

            def emit_pv(j):
                qb, p, kc = iters[j]
                ex = ex_t.pop(j)
                if kc == 0:
                    o_t[0] = ps_acc.tile([128, 512], DT_F32, tag="oacc",
                                         name="o0_t")
                    o_t[1] = ps_acc.tile([128, 512], DT_F32, tag="oacc",
                                         name="o1_t")
                nc.tensor.matmul(
                    o_t[0][0:65, :],
                    lhsT=v_sb[:, kc, 2 * p, :],
                    rhs=ex[:, 0:512],
                    start=(kc == 0), stop=(kc == MQ - 1),
                )
                nc.tensor.matmul(
                    o_t[1][0:65, :],
                    lhsT=v_sb[:, kc, 2 * p + 1, :],
                    rhs=ex[:, 512:1024],
                    start=(kc == 0), stop=(kc == MQ - 1),
                )

            def finish_segment(qb, p):
                """Copy the denominators out now (DVE, cheap); return one
                deferred closure doing broadcast+reciprocal+normalize so the
                PE-side zb matmuls never delay the next segment's QK, and the
                o-slots are freed as early as possible."""
                sl_q = slice(qb * 512, (qb + 1) * 512)
                o0, o1 = o_t[0], o_t[1]
                # one fast copy per head frees the PSUM o-slot immediately;
                # the normalize chain then runs off the SBUF copy at leisure.
                oc0 = work.tile([128, 512], DT_F32, tag="oc", name="oc0_t",
                                bufs=4)
                nc.vector.tensor_copy(oc0[0:65, :], o0[0:65, :])
                oc1 = work.tile([128, 512], DT_F32, tag="oc", name="oc1_t",
                                bufs=4)
                nc.vector.tensor_copy(oc1[0:65, :], o1[0:65, :])

                def go():
                    zb0 = ps_aux.tile([64, 512], DT_F32, tag="aux",
                                      name="zb0_t")
                    nc.tensor.matmul(
                        zb0, lhsT=ones_hi[64:65, 0:64], rhs=oc0[64:65, :],
                        start=True, stop=True)
                    zbi0 = work.tile([64, 512], DT_F32, tag="ot",
                                     name="zbi0_t")
                    nc.vector.reciprocal_approx_fast(out=zbi0, in_=zb0)
                    zb1 = ps_aux.tile([64, 512], DT_F32, tag="aux",
                                      name="zb1_t")
                    nc.tensor.matmul(
                        zb1, lhsT=ones_hi[64:65, 0:64], rhs=oc1[64:65, :],
                        start=True, stop=True)
                    nc.gpsimd.tensor_mul(
                        heads_t[p][0:64, sl_q], oc0[0:64, :], zbi0)
                    zbi1 = work.tile([64, 512], DT_F32, tag="ot",
                                     name="zbi1_t")
                    nc.vector.reciprocal_approx_fast(out=zbi1, in_=zb1)
                    t2 = work.tile([64, 512], DT_BF, tag="t2", name="t2_t")
                    nc.gpsimd.tensor_mul(t2, oc1[0:64, :], zbi1)
                    nc.gpsimd.dma_start(
                        out=heads_t[p][64:128, sl_q], in_=t2)

                return go

            def outproj_ops(m, n2):
                sl_n = slice(n2 * 512, (n2 + 1) * 512)
                st8 = {}

                def mm0():
                    st8["op"] = ps_aux.tile([128, 512], DT_F32, tag="aux",
                                            name="op_t")
                    nc.tensor.matmul(
                        st8["op"],
                        lhsT=heads_t[0][:, m * 128:(m + 1) * 128],
                        rhs=wo_sb[:, 0, sl_n],
                        start=True, stop=False,
                    )

                def mm1():
                    nc.tensor.matmul(
                        st8["op"],
                        lhsT=heads_t[1][:, m * 128:(m + 1) * 128],
                        rhs=wo_sb[:, 1, sl_n],
                        start=False, stop=True,
                    )

                def drain():
                    ob = outs.tile([128, 512], DT_BF, tag="ob", name="ob_t")
                    nc.vector.tensor_copy(ob, st8["op"])
                    nc.sync.dma_start(
                        out=outp[m * 128:(m + 1) * 128, sl_n], in_=ob)

                return [mm0, mm1, drain]

            pA = {}

            def mk_partial0(m, n2):
                def go():
                    opp = ps_aux.tile([128, 512], DT_F32, tag="aux",
                                      name="p0_t")
                    nc.tensor.matmul(
                        opp,
                        lhsT=heads_t[0][:, m * 128:(m + 1) * 128],
                        rhs=wo_sb[:, 0, n2 * 512:(n2 + 1) * 512],
                        start=True, stop=True,
                    )
                    pa = outs.tile([128, 512], DT_F32, tag="pa",
                                   name="pa_t", bufs=8)
                    nc.vector.tensor_copy(pa, opp)
                    pA[m, n2] = pa
                return go

            # preload the exp table while v-projection runs so the switch cost
            # is off the attention critical path
            exwarm = work.tile([1, 16], DT_BF, tag="exw", name="exw_t")
            nc.scalar.activation(exwarm, ones_row[0:1, 0:16],
                                 mybir.ActivationFunctionType.Exp)

            emit_qk(0)
            for j in range(NIT):
                qb, p, kc = iters[j]
                emit_exp(j)
                if j == 0:
                    while q_b:
                        s_, sel_ = q_b.popleft()
                        norm_b(s_, sel_)

                if j + 1 < NIT:
                    emit_qk(j + 1)
                if fillers:
                    fillers.popleft()()
                emit_pv(j)
                if kc == MQ - 1:
                    fillers.insert(0, finish_segment(qb, p))
                    if p == NPAIR - 1 and qb < NQB - 1:
                        for m in range(4 * qb, 4 * qb + 4):
                            for n2 in range(2):
                                fillers.extend(outproj_ops(m, n2))
                    elif p == 0 and qb == NQB - 1:
                        # qb3: heads_t[0]-half of the out-projection runs as
                        # fillers during qb3-p1; partials staged in SBUF so
                        # the tail only needs the heads_t[1] matmul + add.
                        for m in range(4 * qb, 4 * qb + 4):
                            for n2 in range(2):
                                fillers.append(mk_partial0(m, n2))
            while fillers:
                fillers.popleft()()

            # last qb's out-projection: dense back-to-back matmuls through the
            # (now idle) double-buffered sc-tag PSUM slots; drains on the idle
            # ACT engine, DMAs split over two rings.
            for m in range(4 * (NQB - 1), 4 * NQB):
                for n2 in range(2):
                    opp = ps_mm.tile([128, 512], DT_F32, tag="sc",
                                     name="opp_t")
                    nc.tensor.matmul(
                        opp,
                        lhsT=heads_t[1][:, m * 128:(m + 1) * 128],
                        rhs=wo_sb[:, 1, n2 * 512:(n2 + 1) * 512],
                        start=True, stop=True,
                    )
                    ob = outs.tile([128, 512], DT_BF, tag="ob", name="ob_t")
                    nc.vector.tensor_add(ob, opp, pA[m, n2])
                    eng = nc.sync if n2 == 0 else nc.gpsimd
                    eng.dma_start(
                        out=outp[m * 128:(m + 1) * 128,
                                 n2 * 512:(n2 + 1) * 512],
                        in_=ob)

    nc.compile()
    return nc


_CACHE = {}


def _get_program():
    if "nc" not in _CACHE:
        _CACHE["nc"] = build_program()
    return _CACHE["nc"]


def _pmajor(w):
    """[C, N] -> [128, (C//128)*N]: chunk c's rows c*128+p land at
    partition p, free offset c*N — contiguous per-partition DMA lines."""
    C, N = w.shape
    return np.ascontiguousarray(
        w.reshape(C // 128, 128, N).transpose(1, 0, 2).reshape(128, -1)
    ).astype(BF16)


def make_in_maps(query, key, value, in_proj_weight, in_proj_bias,
                 out_proj_weight, out_proj_bias, tau):
    query = np.asarray(query, np.float32)
    key = np.asarray(key, np.float32)
    value = np.asarray(value, np.float32)
    W = np.asarray(in_proj_weight, np.float32)
    bias = np.asarray(in_proj_bias, np.float32)
    Wo = np.asarray(out_proj_weight, np.float32)
    tau_c = np.maximum(np.asarray(tau, np.float32).reshape(H), TAU_MIN)

    # Transposed activations per batch: (E, S) bf16
    xT = {}
    for b in range(B):
        xT["q", b] = np.ascontiguousarray(query[:, b, :].T).astype(BF16)
        xT["k", b] = np.ascontiguousarray(key[:, b, :].T).astype(BF16)
        xT["v", b] = np.ascontiguousarray(value[:, b, :].T).astype(BF16)

    selq_host = np.zeros((2, 128), np.float32)
    selq_host[0, 0:64] = 1.0
    selq_host[1, 64:128] = 1.0
    selq_host = selq_host.astype(BF16)
    in_maps = []
    for c in range(NCORES):
        b = c // 4
        h0 = HPC * (c % 4)
        rows = slice(h0 * HD, (h0 + HPC) * HD)
        rows_k = slice(E + h0 * HD, E + (h0 + HPC) * HD)
        rows_v = slice(2 * E + h0 * HD, 2 * E + (h0 + HPC) * HD)
        # per-pair selector with tau folded in on the k side (the subsequent
        # reciprocal turns tau * |k| into 1 / (tau * |k|))
        selk = np.zeros((2, 256), np.float32)
        for mc in range(NPAIR):
            selk[0, mc * 128:mc * 128 + 64] = tau_c[h0 + 2 * mc]
            selk[1, mc * 128 + 64:(mc + 1) * 128] = tau_c[h0 + 2 * mc + 1]
        in_maps.append({
            "xq_t": xT["q", b],
            "xk_t": xT["k", b],
            "xv_t": xT["v", b],
            "wq_t": _pmajor(W[rows, :].T),
            "wk_t": _pmajor(W[rows_k, :].T),
            "wv_t": _pmajor(W[rows_v, :].T),
            "bq_t": np.ascontiguousarray(
                bias[rows].reshape(2, 128).T).astype(np.float32),
            "bk_t": np.ascontiguousarray(
                bias[rows_k].reshape(2, 128).T).astype(np.float32),
            "b_v": bias[rows_v].reshape(1, 256).astype(BF16),
            "wo_t": _pmajor(Wo[:, rows].T),
            "selk": selk.astype(BF16),
            "selq": selq_host,
        })
    return in_maps


def assemble_out(results, out_proj_bias):
    bo = np.asarray(out_proj_bias, np.float32)
    out = np.zeros((S, B, E), np.float32)
    for c in range(NCORES):
        out[:, c // 4, :] += results[c]["out_p"].astype(np.float32)
    out += bo[None, None, :]
    return out


def kernel(query, key, value, in_proj_weight, in_proj_bias,
           out_proj_weight, out_proj_bias, tau):
    nc = _get_program()
    in_maps = make_in_maps(query, key, value, in_proj_weight, in_proj_bias,
                           out_proj_weight, out_proj_bias, tau)
    res = run_bass_kernel_spmd(nc, in_maps, core_ids=list(range(NCORES)))
    return assemble_out(res.results, out_proj_bias)


if __name__ == "__main__":
    import reference

    inputs = {k: np.asarray(v) for k, v in reference.setup_inputs().items()}
    out = kernel(**inputs)
    print("out shape", out.shape, out.dtype)


# revision 25
# speedup vs baseline: 1.0353x; 1.0060x over previous
"""Cosine multihead attention on 8 Trainium2 NeuronCores.

Sharding: batch*heads across cores. Core c handles batch b = c // 4 and the
4 heads [4*(c%4), 4*(c%4)+4). Each core computes its heads' q/k/v projections
(tensor-parallel slices of in_proj), full attention for its (B,H) slice, and a
partial out-projection (rank-256 contribution). The host sums the 4 partials
per batch and adds out_proj_bias.

Schedule (v2):
- Inputs land via few large rearranged DMAs spread over 4 engine queues so
  compute starts ~3us in and DMA fully overlaps the projection phase.
- All projections (q, k both head-pairs, then v) run first; q/k bias is folded
  into the PSUM drain (tensor_scalar add), killing the bias matmuls.
- Attention runs qb-outer / pair-inner / kc-inner, software-pipelined with the
  QK matmuls emitted one iteration ahead so the ScalarE exp stream (the
  bottleneck: 128 x ~1.1us activations) never waits on the PE.
- Out-projection matmuls are pumped one-per-iteration as PE fillers inside the
  next qb's attention; only the last qb's out-projection trails the loop.
- All sqrts precede all exps: exactly two ACT table loads.
"""

import sys

if "/opt/trn_rl_repo" not in sys.path:
    sys.path.insert(0, "/opt/trn_rl_repo")

from collections import deque

import numpy as np
import ml_dtypes

import concourse.bass as bass
import concourse.tile as tile
from concourse import bacc, mybir
from concourse.bass_utils import run_bass_kernel_spmd

S, B, E, H = 2048, 2, 1024, 16
HD = E // H            # 64
HPC = 4                # heads per core
NCORES = 8
TAU_MIN = 0.01

BF16 = ml_dtypes.bfloat16
DT_BF = mybir.dt.bfloat16
DT_F32 = mybir.dt.float32

KC_E = E // 128        # 8 contraction chunks for projections
MQ = S // 128          # 16 key chunks of 128
NQB = S // 512         # 4 query blocks of 512
NPAIR = HPC // 2       # 2 head pairs per core


def build_program():
    nc = bacc.Bacc(None)

    xq = nc.dram_tensor("xq_t", [E, S], DT_BF, kind="ExternalInput")
    xk = nc.dram_tensor("xk_t", [E, S], DT_BF, kind="ExternalInput")
    xv = nc.dram_tensor("xv_t", [E, S], DT_BF, kind="ExternalInput")
    # weights arrive pre-arranged p-major ([p, c, n]) so their DMAs are
    # fully contiguous 4KB-per-partition lines.
    wq = nc.dram_tensor("wq_t", [128, KC_E * 256], DT_BF, kind="ExternalInput")
    wk = nc.dram_tensor("wk_t", [128, KC_E * 256], DT_BF, kind="ExternalInput")
    wv = nc.dram_tensor("wv_t", [128, KC_E * 256], DT_BF, kind="ExternalInput")
    bqt = nc.dram_tensor("bq_t", [128, 2], DT_F32, kind="ExternalInput")
    bkt = nc.dram_tensor("bk_t", [128, 2], DT_F32, kind="ExternalInput")
    bv = nc.dram_tensor("b_v", [1, 256], DT_BF, kind="ExternalInput")
    wo = nc.dram_tensor("wo_t", [128, 2 * E], DT_BF, kind="ExternalInput")
    selk_in = nc.dram_tensor("selk", [2, 256], DT_BF, kind="ExternalInput")
    selq_in = nc.dram_tensor("selq", [2, 128], DT_BF, kind="ExternalInput")
    outp = nc.dram_tensor("out_p", [S, E], DT_BF, kind="ExternalOutput")

    with tile.TileContext(nc) as tc:
        with (
            tc.tile_pool(name="consts", bufs=1) as consts,
            tc.tile_pool(name="xin", bufs=1) as xin,
            tc.tile_pool(name="wts", bufs=1) as wts,
            tc.tile_pool(name="qk", bufs=1) as qkpool,
            tc.tile_pool(name="vsb", bufs=1) as vpool,
            tc.tile_pool(name="work", bufs=2) as work,
            tc.tile_pool(name="sqp", bufs=4) as sqp,
            tc.tile_pool(name="outs", bufs=3) as outs,
            tc.tile_pool(name="ps_mm", bufs=2, space="PSUM") as ps_mm,
            tc.tile_pool(name="ps_acc", bufs=3, space="PSUM") as ps_acc,
            tc.tile_pool(name="ps_aux", bufs=1, space="PSUM") as ps_aux,
        ):
            # ---- weights / consts: contiguous p-major DMAs on the scalar
            # ring (fast, ~2.5us total), earliest-need first.
            wq_sb = wts.tile([128, KC_E, 256], DT_BF, tag="wq")
            nc.scalar.dma_start(
                out=wq_sb, in_=wq[:, :].rearrange("p (c n) -> p c n", c=KC_E))
            bq_sb = consts.tile([128, 2], DT_F32, tag="bq")
            nc.scalar.dma_start(out=bq_sb, in_=bqt[:, :])
            selq = consts.tile([2, 128], DT_BF, tag="selq")
            nc.scalar.dma_start(out=selq, in_=selq_in[:, :])
            wk_sb = wts.tile([128, KC_E, 256], DT_BF, tag="wk")
            nc.scalar.dma_start(
                out=wk_sb, in_=wk[:, :].rearrange("p (c n) -> p c n", c=KC_E))
            bk_sb = consts.tile([128, 2], DT_F32, tag="bk")
            nc.scalar.dma_start(out=bk_sb, in_=bkt[:, :])
            selk_sb = consts.tile([2, 256], DT_BF, tag="selk")
            nc.scalar.dma_start(out=selk_sb, in_=selk_in[:, :])
            wv_sb = wts.tile([128, KC_E, 256], DT_BF, tag="wv")
            nc.scalar.dma_start(
                out=wv_sb, in_=wv[:, :].rearrange("p (c n) -> p c n", c=KC_E))
            bv_sb = consts.tile([1, 256], DT_BF, tag="bv")
            nc.scalar.dma_start(out=bv_sb, in_=bv[:, :])
            wo_sb = wts.tile([128, 2, E], DT_BF, tag="wo")
            nc.scalar.dma_start(
                out=wo_sb, in_=wo[:, :].rearrange("p (c n) -> p c n", c=2))

            ones_row = consts.tile([1, 512], DT_BF, tag="ones_row")
            nc.vector.memset(ones_row, 1.0)
            ones_hi = consts.tile([128, 64], DT_F32, tag="ones_hi")
            nc.vector.memset(ones_hi, 1.0)
            hsel = consts.tile([128, 2], DT_BF, tag="hsel")
            nc.vector.memset(hsel, 0.0)
            nc.vector.memset(hsel[0:64, 0:1], 1.0)
            nc.vector.memset(hsel[64:128, 1:2], 1.0)

            # ---- activations: each tensor's 4 chunk-group DMAs spread over
            # all three DMA rings (sync/gpsimd/scalar) so per-tensor
            # completion is ~3x faster; tensor order xq -> xk -> xv matches
            # consumption order.
            xq_sb = xin.tile([128, KC_E, S], DT_BF, tag="xq")
            xk_sb = xin.tile([128, KC_E, S], DT_BF, tag="xk")
            xv_sb = xin.tile([128, KC_E, S], DT_BF, tag="xv")
            for t_sb, t_dram, g0, rings in (
                    (xq_sb, xq, 0, (nc.sync, nc.gpsimd, nc.sync, nc.gpsimd)),
                    (xv_sb, xv, 0, (nc.gpsimd, nc.sync, nc.gpsimd, nc.sync)),
                    (xk_sb, xk, 0, (nc.scalar, nc.sync, nc.scalar,
                                    nc.gpsimd))):
                for g in range(g0, 4):
                    src = t_dram[g * 256:(g + 1) * 256, :].rearrange(
                        "(c p) s -> p c s", p=128)
                    rings[g].dma_start(
                        out=t_sb[:, 2 * g:2 * g + 2, :], in_=src)

            qt = [qkpool.tile([128, S], DT_BF, tag=f"qt{p}", name=f"qt{p}")
                  for p in range(NPAIR)]
            kt = [qkpool.tile([128, S], DT_BF, tag=f"kt{p}", name=f"kt{p}")
                  for p in range(NPAIR)]
            heads_t = [qkpool.tile([128, S], DT_BF, tag=f"ht{p}", name=f"ht{p}")
                       for p in range(NPAIR)]

            # ---- q/k projection: 8 accum matmuls per unit, bias folded into
            # the PSUM drain. The L2-norm is pipelined two units deep so the
            # PE never waits on the ACT sqrt: unit u's proj matmuls run while
            # u-1's sumsq->sqrt and u-2's broadcast->scale complete.
            def proj_mms(dst, w_sb, b_col, x_sb, mc, n4):
                sl = slice(n4 * 512, (n4 + 1) * 512)
                pp = ps_mm.tile([128, 512], DT_F32, tag="sc", name="pp_t")
                for c in range(KC_E):
                    nc.tensor.matmul(
                        pp,
                        lhsT=w_sb[:, c, mc * 128:(mc + 1) * 128],
                        rhs=x_sb[:, c, sl],
                        start=(c == 0),
                        stop=(c == KC_E - 1),
                    )
                # drain + bias on ACT (Identity folds the per-partition bias)
                nc.scalar.activation(
                    dst[:, sl], pp, mybir.ActivationFunctionType.Identity,
                    bias=b_col)
                sq = sqp.tile([128, 512], DT_BF, tag="sq", name="sq_t")
                nc.gpsimd.tensor_mul(sq, dst[:, sl], dst[:, sl])
                return (dst, sl, sq)

            def norm_a(stg):
                dst, sl, sq = stg
                ss = ps_acc.tile([2, 512], DT_F32, tag="oacc", name="ss_t")
                nc.tensor.matmul(ss, lhsT=hsel, rhs=sq, start=True, stop=True)
                st = work.tile([2, 512], DT_BF, tag="st", name="st_t")
                nc.scalar.activation(st, ss, mybir.ActivationFunctionType.Sqrt)
                return (dst, sl, st)

            def norm_b(stg, sel):
                dst, sl, st = stg
                rb = ps_aux.tile([128, 512], DT_F32, tag="aux", name="rb_t")
                nc.tensor.matmul(rb, lhsT=sel, rhs=st, start=True, stop=True)
                rbi = work.tile([128, 512], DT_F32, tag="rbi", name="rbi_t")
                nc.vector.reciprocal_approx_fast(out=rbi, in_=rb)
                nc.gpsimd.tensor_mul(dst[:, sl], dst[:, sl], rbi)

            q_units = [
                (qt[mc], wq_sb, bq_sb[:, mc:mc + 1], xq_sb, selq, mc, n4)
                for mc in range(NPAIR) for n4 in range(4)
            ]
            k_units = [
                (kt[mc], wk_sb, bk_sb[:, mc:mc + 1], xk_sb,
                 selk_sb[:, mc * 128:(mc + 1) * 128], mc, n4)
                for mc in range(NPAIR) for n4 in range(4)
            ]
            q_a, q_b = deque(), deque()

            def pump_unit(unit):
                dst, w_sb, b_col, x_sb, sel, mc, n4 = unit
                stg = proj_mms(dst, w_sb, b_col, x_sb, mc, n4)
                q_a.append((stg, sel))
                if len(q_a) >= 2:
                    s, sel_ = q_a.popleft()
                    q_b.append((norm_a(s), sel_))
                if len(q_b) >= 2:
                    s, sel_ = q_b.popleft()
                    norm_b(s, sel_)

            def flush_units():
                while q_a:
                    s, sel_ = q_a.popleft()
                    q_b.append((norm_a(s), sel_))
                while q_b:
                    s, sel_ = q_b.popleft()
                    norm_b(s, sel_)

            for unit in q_units:
                pump_unit(unit)

            # ---- v projection between q and k (natural orientation, ones
            # column for the softmax denominator ride-along in PV); the q-norm
            # pipeline flushes underneath the first v matmuls, and attention's
            # progressive kt consumption later overlaps the k-proj tail.
            v_sb = vpool.tile([128, MQ, HPC, HD + 1], DT_BF, tag="v")
            nc.vector.memset(v_sb[:, :, :, HD:HD + 1], 1.0)
            for m in range(MQ):
                vp = ps_acc.tile([128, 256], DT_F32, tag="oacc", name="vp_t")
                for c in range(KC_E):
                    nc.tensor.matmul(
                        vp,
                        lhsT=xv_sb[:, c, m * 128:(m + 1) * 128],
                        rhs=wv_sb[:, c, :],
                        start=(c == 0),
                        stop=False,
                    )
                nc.tensor.matmul(
                    vp,
                    lhsT=ones_row[0:1, 0:128],
                    rhs=bv_sb[0:1, :],
                    start=False,
                    stop=True,
                )
                nc.vector.tensor_copy(
                    out=v_sb[:, m, :, 0:HD],
                    in_=vp.rearrange("p (h d) -> p h d", h=HPC),
                )
                if m < 2 and q_a:
                    s, sel_ = q_a.popleft()
                    q_b.append((norm_a(s), sel_))
                elif q_b:
                    s, sel_ = q_b.popleft()
                    norm_b(s, sel_)

            for unit in k_units:
                pump_unit(unit)
            flush_units()

            # ---- attention: software-pipelined over (qb, pair, kc) ---------
            iters = [(qb, p, kc)
                     for qb in range(NQB) for p in range(NPAIR)
                     for kc in range(MQ)]
            NIT = len(iters)
            sc_t, ex_t, o_t = {}, {}, {}
            fillers = deque()

            def emit_qk(j):
                qb, p, kc = iters[j]
                sl_q = slice(qb * 512, (qb + 1) * 512)
                sc = ps_mm.tile([128, 1024], DT_F32, tag="sc", name="sc_t")
                nc.tensor.matmul(
                    sc[:, 0:512],
                    lhsT=kt[p][0:64, kc * 128:(kc + 1) * 128],
                    rhs=qt[p][0:64, sl_q],
                    start=True, stop=True,
                )
                nc.tensor.matmul(
                    sc[:, 512:1024],
                    lhsT=kt[p][64:128, kc * 128:(kc + 1) * 128],
                    rhs=qt[p][64:128, sl_q],
                    start=True, stop=True,
                )
                sc_t[j] = sc

            def emit_exp(j):
                sc = sc_t.pop(j)
                ex = work.tile([128, 1024], DT_BF, tag="exp", name="ex_t",
                               bufs=3)
                nc.scalar.activation(ex, sc, mybir.ActivationFunctionType.Exp)
                ex_t[j] = ex

            def emit_pv(j):
                qb, p, kc = iters[j]
                ex = ex_t.pop(j)
                if kc == 0:
                    o_t[0] = ps_acc.tile([128, 512], DT_F32, tag="oacc",
                                         name="o0_t")
                    o_t[1] = ps_acc.tile([128, 512], DT_F32, tag="oacc",
                                         name="o1_t")
                nc.tensor.matmul(
                    o_t[0][0:65, :],
                    lhsT=v_sb[:, kc, 2 * p, :],
                    rhs=ex[:, 0:512],
                    start=(kc == 0), stop=(kc == MQ - 1),
                )
                nc.tensor.matmul(
                    o_t[1][0:65, :],
                    lhsT=v_sb[:, kc, 2 * p + 1, :],
                    rhs=ex[:, 512:1024],
                    start=(kc == 0), stop=(kc == MQ - 1),
                )

            def finish_segment(qb, p):
                """Copy the denominators out now (DVE, cheap); return one
                deferred closure doing broadcast+reciprocal+normalize so the
                PE-side zb matmuls never delay the next segment's QK, and the
                o-slots are freed as early as possible."""
                sl_q = slice(qb * 512, (qb + 1) * 512)
                o0, o1 = o_t[0], o_t[1]
                # one fast copy per head frees the PSUM o-slot immediately;
                # the normalize chain then runs off the SBUF copy at leisure.
                oc0 = work.tile([128, 512], DT_F32, tag="oc", name="oc0_t",
                                bufs=4)
                nc.vector.tensor_copy(oc0[0:65, :], o0[0:65, :])
                oc1 = work.tile([128, 512], DT_F32, tag="oc", name="oc1_t",
                                bufs=4)
                nc.vector.tensor_copy(oc1[0:65, :], o1[0:65, :])

                def go():
                    zb0 = ps_aux.tile([64, 512], DT_F32, tag="aux",
                                      name="zb0_t")
                    nc.tensor.matmul(
                        zb0, lhsT=ones_hi[64:65, 0:64], rhs=oc0[64:65, :],
                        start=True, stop=True)
                    zbi0 = work.tile([64, 512], DT_F32, tag="ot",
                                     name="zbi0_t")
                    nc.vector.reciprocal_approx_fast(out=zbi0, in_=zb0)
                    zb1 = ps_aux.tile([64, 512], DT_F32, tag="aux",
                                      name="zb1_t")
                    nc.tensor.matmul(
                        zb1, lhsT=ones_hi[64:65, 0:64], rhs=oc1[64:65, :],
                        start=True, stop=True)
                    nc.gpsimd.tensor_mul(
                        heads_t[p][0:64, sl_q], oc0[0:64, :], zbi0)
                    zbi1 = work.tile([64, 512], DT_F32, tag="ot",
                                     name="zbi1_t")
                    nc.vector.reciprocal_approx_fast(out=zbi1, in_=zb1)
                    t2 = work.tile([64, 512], DT_BF, tag="t2", name="t2_t")
                    nc.gpsimd.tensor_mul(t2, oc1[0:64, :], zbi1)
                    nc.gpsimd.dma_start(
                        out=heads_t[p][64:128, sl_q], in_=t2)

                return go

            def outproj_ops(m, n2):
                sl_n = slice(n2 * 512, (n2 + 1) * 512)
                st8 = {}

                def mm0():
                    st8["op"] = ps_aux.tile([128, 512], DT_F32, tag="aux",
                                            name="op_t")
                    nc.tensor.matmul(
                        st8["op"],
                        lhsT=heads_t[0][:, m * 128:(m + 1) * 128],
                        rhs=wo_sb[:, 0, sl_n],
                        start=True, stop=False,
                    )

                def mm1():
                    nc.tensor.matmul(
                        st8["op"],
                        lhsT=heads_t[1][:, m * 128:(m + 1) * 128],
                        rhs=wo_sb[:, 1, sl_n],
                        start=False, stop=True,
                    )

                def drain():
                    ob = outs.tile([128, 512], DT_BF, tag="ob", name="ob_t")
                    nc.vector.tensor_copy(ob, st8["op"])
                    nc.sync.dma_start(
                        out=outp[m * 128:(m + 1) * 128, sl_n], in_=ob)

                return [mm0, mm1, drain]

            pA = {}

            def mk_partial0(m, n2):
                def go():
                    opp = ps_aux.tile([128, 512], DT_F32, tag="aux",
                                      name="p0_t")
                    nc.tensor.matmul(
                        opp,
                        lhsT=heads_t[0][:, m * 128:(m + 1) * 128],
                        rhs=wo_sb[:, 0, n2 * 512:(n2 + 1) * 512],
                        start=True, stop=True,
                    )
                    pa = outs.tile([128, 512], DT_F32, tag="pa",
                                   name="pa_t", bufs=8)
                    nc.vector.tensor_copy(pa, opp)
                    pA[m, n2] = pa
                return go

            # preload the exp table while v-projection runs so the switch cost
            # is off the attention critical path
            exwarm = work.tile([1, 16], DT_BF, tag="exw", name="exw_t")
            nc.scalar.activation(exwarm, ones_row[0:1, 0:16],
                                 mybir.ActivationFunctionType.Exp)

            emit_qk(0)
            for j in range(NIT):
                qb, p, kc = iters[j]
                emit_exp(j)

                if j + 1 < NIT:
                    emit_qk(j + 1)
                if fillers:
                    fillers.popleft()()
                emit_pv(j)
                if kc == MQ - 1:
                    fillers.insert(0, finish_segment(qb, p))
                    if p == NPAIR - 1 and qb < NQB - 1:
                        for m in range(4 * qb, 4 * qb + 4):
                            for n2 in range(2):
                                fillers.extend(outproj_ops(m, n2))
                    elif p == 0 and qb == NQB - 1:
                        # qb3: heads_t[0]-half of the out-projection runs as
                        # fillers during qb3-p1; partials staged in SBUF so
                        # the tail only needs the heads_t[1] matmul + add.
                        for m in range(4 * qb, 4 * qb + 4):
                            for n2 in range(2):
                                fillers.append(mk_partial0(m, n2))
            while fillers:
                fillers.popleft()()

            # last qb's out-projection: dense back-to-back matmuls through the
            # (now idle) double-buffered sc-tag PSUM slots; drains on the idle
            # ACT engine, DMAs split over two rings.
            for m in range(4 * (NQB - 1), 4 * NQB):
                for n2 in range(2):
                    opp = ps_mm.tile([128, 512], DT_F32, tag="sc",
                                     name="opp_t")
                    nc.tensor.matmul(
                        opp,
                        lhsT=heads_t[1][:, m * 128:(m + 1) * 128],
                        rhs=wo_sb[:, 1, n2 * 512:(n2 + 1) * 512],
                        start=True, stop=True,
                    )
                    ob = outs.tile([128, 512], DT_BF, tag="ob", name="ob_t")
                    nc.vector.tensor_add(ob, opp, pA[m, n2])
                    eng = nc.sync if n2 == 0 else nc.gpsimd
                    eng.dma_start(
                        out=outp[m * 128:(m + 1) * 128,
                                 n2 * 512:(n2 + 1) * 512],
                        in_=ob)

    nc.compile()
    return nc


_CACHE = {}


def _get_program():
    if "nc" not in _CACHE:
        _CACHE["nc"] = build_program()
    return _CACHE["nc"]


def _pmajor(w):
    """[C, N] -> [128, (C//128)*N]: chunk c's rows c*128+p land at
    partition p, free offset c*N — contiguous per-partition DMA lines."""
    C, N = w.shape
    return np.ascontiguousarray(
        w.reshape(C // 128, 128, N).transpose(1, 0, 2).reshape(128, -1)
    ).astype(BF16)


def make_in_maps(query, key, value, in_proj_weight, in_proj_bias,
                 out_proj_weight, out_proj_bias, tau):
    query = np.asarray(query, np.float32)
    key = np.asarray(key, np.float32)
    value = np.asarray(value, np.float32)
    W = np.asarray(in_proj_weight, np.float32)
    bias = np.asarray(in_proj_bias, np.float32)
    Wo = np.asarray(out_proj_weight, np.float32)
    tau_c = np.maximum(np.asarray(tau, np.float32).reshape(H), TAU_MIN)

    # Transposed activations per batch: (E, S) bf16
    xT = {}
    for b in range(B):
        xT["q", b] = np.ascontiguousarray(query[:, b, :].T).astype(BF16)
        xT["k", b] = np.ascontiguousarray(key[:, b, :].T).astype(BF16)
        xT["v", b] = np.ascontiguousarray(value[:, b, :].T).astype(BF16)

    selq_host = np.zeros((2, 128), np.float32)
    selq_host[0, 0:64] = 1.0
    selq_host[1, 64:128] = 1.0
    selq_host = selq_host.astype(BF16)
    in_maps = []
    for c in range(NCORES):
        b = c // 4
        h0 = HPC * (c % 4)
        rows = slice(h0 * HD, (h0 + HPC) * HD)
        rows_k = slice(E + h0 * HD, E + (h0 + HPC) * HD)
        rows_v = slice(2 * E + h0 * HD, 2 * E + (h0 + HPC) * HD)
        # per-pair selector with tau folded in on the k side (the subsequent
        # reciprocal turns tau * |k| into 1 / (tau * |k|))
        selk = np.zeros((2, 256), np.float32)
        for mc in range(NPAIR):
            selk[0, mc * 128:mc * 128 + 64] = tau_c[h0 + 2 * mc]
            selk[1, mc * 128 + 64:(mc + 1) * 128] = tau_c[h0 + 2 * mc + 1]
        in_maps.append({
            "xq_t": xT["q", b],
            "xk_t": xT["k", b],
            "xv_t": xT["v", b],
            "wq_t": _pmajor(W[rows, :].T),
            "wk_t": _pmajor(W[rows_k, :].T),
            "wv_t": _pmajor(W[rows_v, :].T),
            "bq_t": np.ascontiguousarray(
                bias[rows].reshape(2, 128).T).astype(np.float32),
            "bk_t": np.ascontiguousarray(
                bias[rows_k].reshape(2, 128).T).astype(np.float32),
            "b_v": bias[rows_v].reshape(1, 256).astype(BF16),
            "wo_t": _pmajor(Wo[:, rows].T),
            "selk": selk.astype(BF16),
            "selq": selq_host,
        })
    return in_maps


def assemble_out(results, out_proj_bias):
    bo = np.asarray(out_proj_bias, np.float32)
    out = np.zeros((S, B, E), np.float32)
    for c in range(NCORES):
        out[:, c // 4, :] += results[c]["out_p"].astype(np.float32)
    out += bo[None, None, :]
    return out


def kernel(query, key, value, in_proj_weight, in_proj_bias,
           out_proj_weight, out_proj_bias, tau):
    nc = _get_program()
    in_maps = make_in_maps(query, key, value, in_proj_weight, in_proj_bias,
                           out_proj_weight, out_proj_bias, tau)
    res = run_bass_kernel_spmd(nc, in_maps, core_ids=list(range(NCORES)))
    return assemble_out(res.results, out_proj_bias)


if __name__ == "__main__":
    import reference

    inputs = {k: np.asarray(v) for k, v in reference.setup_inputs().items()}
    out = kernel(**inputs)
    print("out shape", out.shape, out.dtype)
